# revision 78
# baseline (speedup 1.0000x reference)
"""Trainium2 Bass kernel v3 for the dense transformer block.

Distribution (zero-collective): data-parallel over batch (B=2) x
query-row-parallel over token tiles (4 ways) => 8 cores. Each core
computes K/V (all heads, full T) and Q (own 512 rows, gathered
round-robin) and produces the final output for its 4 q-row tiles.

One program for all 8 cores: per-core differences (which q rows, the
causal/text stripe masks) are carried by input data. With q-tile j of
core c at absolute tile (c%4)+4j, k-tile kt is strictly above the
diagonal for j < kt//4 (skipped), fully allowed for j > kt//4, and the
single stripe j == kt//4 gets a data-driven 0/1 mask multiplied in
after exp.

Precision: fp8e4 + DoubleRow (2x PE) for the QKV projections, the
attention AV product (exp probs + V in fp8, exp shifted by -2 so it
cannot overflow fp8's 448 max) and the attention out-projection
(descaled 1/256 into the residual); bf16 for scores and the MLP (fp8
there fails the 2e-2 gate); fp32 PSUM accumulation and residuals.

Phase A: x is loaded bf16 for non-own tiles (norm-only use) and f32
for own rows (residual); the 1/rms norm scale is folded into the rope
tables and the V psum copy so the PE transposes depend only on the x
DMA. Rope tables are compact (T,64) and stride-0-broadcast across
heads. RoPE combines run on GpSimd, mults on DVE from PSUM.

Phase B: per-head softmax denominator 1/D computed as exp(-ln D) on
the scalar engine (the DVE reciprocal on a (1,512) row costs 3.4us on
one lane and stalled the PE); the PE denominator-broadcast + normalize
of head h are deferred into head h+1's score stream, and each block's
AV matmuls are deferred one ktp block behind its scores.

DMA: x/tables on the sync queue, K/V weight halves split across the
scalar+gpsimd queues, MLP weights w1/sync w2/sync wd/gpsimd with the
first down-proj blocks prefetched during phase D. No DRAM scratch.
"""

import numpy as np
import ml_dtypes

import concourse.bass as bass
import concourse.mybir as mybir
import concourse.tile as tile
from concourse.bass_utils import run_bass_kernel_spmd
from concourse.masks import make_identity
from concourse.vector_clock import ScopedClock

# ---------------------------------------------------------------- shapes
B = 2
T = 2048
C = 1024
NH = 16
HD = 64
NHID = 2816
EPS = 1e-5
P = 128
TTILES = T // P      # 16
QTILES = 4           # q-row tiles per core
QROWS = QTILES * P   # 512
CCH = C // P         # 8 contraction chunks
CCP = CCH // 2       # 4 DoubleRow pairs
HSB = NHID // P      # 22 hidden blocks
F32 = mybir.dt.float32
BF16 = mybir.dt.bfloat16
F8 = mybir.dt.float8e4
WS = 16.0            # fp8 weight scale
EXPSCALE = 0.125 / (WS * WS)

AF = mybir.ActivationFunctionType
DR = mybir.MatmulPerfMode.DoubleRow

# ------------------------------------------------- TileContext drain patch
_DRAIN_CAP = 1


def _patched_drain_and_barrier(self, tick_clock, wait_clock):
    nc = self.nc
    drain_inst = nc.sync.drain()
    wait_clock.add_sem_waits(
        drain_inst.ins, ScopedClock({None: tick_clock.global_clock})
    )
    waits = list(drain_inst.ins.sync_info.on_wait)
    if len(waits) > _DRAIN_CAP:
        upd = list(drain_inst.ins.sync_info.on_update)
        drain_inst.ins.sync_info = mybir.SyncInfo(
            on_wait=waits[:_DRAIN_CAP], on_update=upd
        )
        rest = waits[_DRAIN_CAP:]
        while rest:
            d2 = nc.sync.drain()
            d2.ins.sync_info = mybir.SyncInfo(
                on_wait=rest[:_DRAIN_CAP], on_update=[]
            )
            rest = rest[_DRAIN_CAP:]
    nc.all_engine_barrier()
    popped = nc._tile_sem_poison_stack.pop()
    assert popped is self._sem_poison
    nc.clear_and_free_semaphores(list(self.sems.allocated().values()))
    nc.all_engine_barrier()


tile.TileContext._drain_and_barrier = _patched_drain_and_barrier

_JSON_WAIT_CAP = 1
_WAIT_CAPS = {}
_WAIT_CAP_DEFAULT = 1


def _split_bir_waits(bir_bytes):
    import orjson
    d = orjson.loads(bir_bytes)
    n_split = 0
    for fn in d["functions"]:
        for blk in fn["blocks"]:
            out = []
            for inst in blk["instructions"]:
                si = inst.get("sync_info")
                waits = (si or {}).get("on_wait") or []
                cap = _WAIT_CAPS.get(inst.get("opcode"), _WAIT_CAP_DEFAULT)
                if len(waits) > cap:
                    keep = waits[-cap:]
                    extra = waits[:-cap]
                    w_i = 0
                    while extra:
                        chunk = extra[:_JSON_WAIT_CAP]
                        extra = extra[_JSON_WAIT_CAP:]
                        nop = {
                            "name": f"{inst['name']}_w{w_i}",
                            "opcode": "NoOp",
                            "engine": inst["engine"],
                            "ins": [],
                            "outs": [],
                            "sync_info": {"on_wait": chunk, "on_update": []},
                        }
                        if "debug" in inst:
                            nop["debug"] = inst["debug"]
                        out.append(nop)
                        w_i += 1
                    si["on_wait"] = keep
                    n_split += 1
                out.append(inst)
            blk["instructions"] = out
    return orjson.dumps(d), n_split


import concourse.bass_utils as _bass_utils_mod
import concourse.bass2jax as _bass2jax_mod

_orig_compile_bir_kernel = _bass_utils_mod.compile_bir_kernel


def _patched_compile_bir_kernel(ant_bir_str, compile_dir_path, **kwargs):
    fixed, n = _split_bir_waits(ant_bir_str)
    return _orig_compile_bir_kernel(fixed, compile_dir_path, **kwargs)


_bass_utils_mod.compile_bir_kernel = _patched_compile_bir_kernel
_bass2jax_mod.compile_bir_kernel = _patched_compile_bir_kernel




# ------------------------------------------------------------ device code
def _emit(tc, nc, prm, upto="full"):
    from contextlib import ExitStack

    def _dummy_out():
        nc.sync.dma_start(out=prm["out"][:, :], in_=prm["xown"][:, :])

    with ExitStack() as es:
        constp = es.enter_context(tc.tile_pool(name="const", bufs=1))
        identity_f = constp.tile([P, P], F32, tag="idf", name="idf")
        make_identity(nc, identity_f)
        identity = constp.tile([P, P], BF16, tag="idb", name="idb")
        nc.vector.tensor_copy(identity, identity_f)
        eps_t = constp.tile([P, 1], F32, tag="eps", name="eps")
        nc.vector.memset(eps_t, EPS)
        nshift = constp.tile([P, 1], F32, tag="nshift", name="nshift")
        nc.vector.memset(nshift, -2.0)
        ones_bf = constp.tile([P, HD], BF16, tag="ones", name="ones")
        nc.vector.memset(ones_bf, 1.0)
        smask = constp.tile([P, TTILES, P], F8, tag="smask", name="smask")

        yT2p = es.enter_context(tc.tile_pool(name="yT2", bufs=1))
        yT2m = yT2p.tile([P, CCH, QROWS], F8, tag="yT2m", name="yT2m")
        xqp = es.enter_context(tc.tile_pool(name="xq", bufs=1))
        xq_sb = [xqp.tile([P, C], F32, tag=f"xq{j}", name=f"xq{j}")
                 for j in range(QTILES)]
        wpjp = es.enter_context(tc.tile_pool(name="wpj", bufs=1))
        wpj = wpjp.tile([P, CCH, C], F8, tag="wpj", name="wpj")
        # attention SBUF arrays: kept alive to the end (SBUF fits) so
        # the B->C transition needs no drain+barrier pair
        attnp = es.enter_context(tc.tile_pool(name="attnbig", bufs=1))
        KTm = attnp.tile([P, CCH, T], BF16, tag="KTm", name="KTm")
        # Q in two head-parity copies, the other head's partitions
        # zeroed: scores then use full-128-partition operands (64-high
        # matmuls stream at ~half the PE column rate)
        QTmE = attnp.tile([P, CCH, QROWS], BF16, tag="QTmE", name="QTmE")
        QTmO = attnp.tile([P, CCH, QROWS], BF16, tag="QTmO", name="QTmO")
        nc.gpsimd.memset(QTmE, 0.0)
        nc.gpsimd.memset(QTmO, 0.0)
        Vm = attnp.tile([P, TTILES, NH, HD + 1], F8, tag="Vm", name="Vm")
        hTm = attnp.tile([P, CCH, T], F8, tag="hTm", name="hTm")

        # ---------------- Phase A: norm + QKV + rope + transposes --------
        if True:
            with tc.tile_pool(name="w8", bufs=1) as w8p, \
                 tc.tile_pool(name="pa", bufs=3) as pa, \
                 tc.tile_pool(name="stat", bufs=6) as statp, \
                 tc.tile_pool(name="rope", bufs=4) as ropep, \
                 tc.tile_pool(name="tab", bufs=3) as tabp, \
                 tc.tile_pool(name="qkps", bufs=4, space="PSUM") as qkpp, \
                 tc.tile_pool(name="tpps", bufs=2, space="PSUM") as tppp:
                # startup order: tile-0 data, K weights, more tiles, V/Q
                pref = {}

                def pref_tile(tt):
                    # non-own tiles feed only rmsnorm -> K/V: bf16 x is
                    # plenty (fp8 QKV quantization dominates)
                    xt = pa.tile([P, C], BF16, tag="xt", name="xt")
                    nc.sync.dma_start(
                        out=xt, in_=prm["xbf"][tt * P:(tt + 1) * P, :])
                    ct = tabp.tile([P, 64], BF16, tag="cos", name="cos")
                    st = tabp.tile([P, 64], BF16, tag="sin", name="sin")
                    nc.sync.dma_start(
                        out=ct, in_=prm["cose"][tt * P:(tt + 1) * P, :])
                    nc.sync.dma_start(
                        out=st, in_=prm["sine"][tt * P:(tt + 1) * P, :])
                    pref[tt] = (xt, ct, st)

                pref_tile(0)
                wq8 = w8p.tile([P, CCH, C], F8, tag="wq8", name="wq8")
                wk8 = w8p.tile([P, CCH, C], F8, tag="wk8", name="wk8")
                wv8 = w8p.tile([P, CCH, C], F8, tag="wv8", name="wv8")
                # split K/V weight loads by fc half across idle queues so
                # the first projections start ~4us earlier
                nc.scalar.dma_start(
                    out=wk8[:, :, 0:512],
                    in_=prm["wk8"][:, :, 0:512].rearrange(
                        "cc p o -> p cc o"))
                nc.gpsimd.dma_start(
                    out=wk8[:, :, 512:1024],
                    in_=prm["wk8"][:, :, 512:1024].rearrange(
                        "cc p o -> p cc o"))
                nc.scalar.dma_start(
                    out=wv8[:, :, 0:512],
                    in_=prm["wv8"][:, :, 0:512].rearrange(
                        "cc p o -> p cc o"))
                nc.gpsimd.dma_start(
                    out=wv8[:, :, 512:1024],
                    in_=prm["wv8"][:, :, 512:1024].rearrange(
                        "cc p o -> p cc o"))
                nc.gpsimd.dma_start(
                    out=wq8, in_=prm["wq8"].rearrange("cc p o -> p cc o"))
                nc.gpsimd.dma_start(
                    out=smask, in_=prm["smask"].rearrange("kt p q -> p kt q"))
                pref_tile(1)
                pref_tile(2)
                scratch = pa.tile([P, C], F32, tag="sq_scr", name="sq_scr",
                                  bufs=1)

                def norm_to(xt, dstT, tcol, tag):
                    """Norm stats + transpose of RAW x into fp8
                    dstT[:, :, tcol:tcol+128]. The 1/rms scale is NOT
                    applied here: it folds into the rope tables (K/Q)
                    and the V psum copy, so the PE transposes depend
                    only on the x DMA, not the scalar norm chain.
                    Returns the (128,1) 1/rms tile."""
                    ssq = statp.tile([P, 1], F32, tag=f"{tag}ssq",
                                     name=f"{tag}ssq")
                    nc.scalar.activation(out=scratch, in_=xt,
                                         func=AF.Square, accum_out=ssq)
                    f = statp.tile([P, 1], F32, tag=f"{tag}f",
                                   name=f"{tag}f")
                    nc.scalar.activation(out=f, in_=ssq, func=AF.Sqrt,
                                         bias=eps_t, scale=1.0 / C)
                    nc.vector.reciprocal(f, f)
                    if xt.dtype != BF16:
                        xb = pa.tile([P, C], BF16, tag="xb", name="xb")
                        nc.gpsimd.tensor_copy(xb, xt)
                        src = xb
                    else:
                        src = xt
                    tpw = tppp.tile([P, CCH, P], BF16, tag="tpwh",
                                    name="tpwh")
                    for dc in range(CCH):
                        nc.tensor.transpose(
                            tpw[:, dc, :], src[:, dc * P:(dc + 1) * P],
                            identity)
                    nc.scalar.activation(
                        out=dstT[:, :, tcol:tcol + P], in_=tpw,
                        func=AF.Copy)
                    return f

                def project(dst_psum, srcT, scol, wsb, fc):
                    """QKV matmuls: (128 t-rows at srcT col scol) x
                    (feature chunk fc) into dst_psum (128, 512).
                    fp8 DoubleRow: two contraction chunks per pass."""
                    for cp in range(CCP):
                        nc.tensor.matmul(
                            dst_psum,
                            lhsT=srcT[:, 2 * cp:2 * cp + 2, scol:scol + P],
                            rhs=wsb[:, 2 * cp:2 * cp + 2,
                                    fc * 512:(fc + 1) * 512],
                            start=(cp == 0), stop=(cp == CCP - 1),
                            perf_mode=DR,
                        )

                def rope_tp(psums, ct, st, dstT, tcol, tag):
                    """RoPE from two (128,512) psum chunks + transpose into
                    dstT[:, :, tcol:tcol+128] (bf16). Split by column half
                    across DVE and GpSimd (GpSimd idles otherwise and the
                    rope chain is the phase-A critical path)."""
                    A = ropep.tile([P, C], BF16, tag="rA", name=f"{tag}A")
                    Bt = ropep.tile([P, C], BF16, tag="rB", name=f"{tag}B")
                    # compact (128,64) tables, stride-0-broadcast across
                    # the 8 heads of each 512-col psum chunk
                    ctb = ct.unsqueeze(1).to_broadcast([P, 8, 64])
                    stb = st.unsqueeze(1).to_broadcast([P, 8, 64])
                    A4 = A.rearrange("p (r f) -> p r f", f=64)
                    B4 = Bt.rearrange("p (r f) -> p r f", f=64)
                    for fc in range(2):
                        hsl = slice(fc * 8, (fc + 1) * 8)
                        ps4 = psums[fc].rearrange("p (r f) -> p r f", f=64)
                        nc.vector.tensor_mul(A4[:, hsl, :], ps4, ctb)
                        nc.vector.tensor_mul(B4[:, hsl, :], ps4, stb)
                    kr = ropep.tile([P, C], BF16, tag="rr", name=f"{tag}r")
                    A3 = A.rearrange("p (i two) -> p i two", two=2)
                    B3 = Bt.rearrange("p (i two) -> p i two", two=2)
                    k3 = kr.rearrange("p (i two) -> p i two", two=2)
                    # combines are SBUF-only: run them on GpSimd (PSUM is
                    # not GpSimd-accessible) to shorten the DVE chain
                    nc.gpsimd.tensor_sub(k3[:, :, 0], A3[:, :, 0],
                                         B3[:, :, 1])
                    nc.gpsimd.tensor_add(k3[:, :, 1], A3[:, :, 1],
                                         B3[:, :, 0])
                    tpw = tppp.tile([P, CCH, P], BF16, tag="tpw",
                                    name="tpw")
                    for dc in range(CCH):
                        nc.tensor.transpose(
                            tpw[:, dc, :], kr[:, dc * P:(dc + 1) * P],
                            identity)
                    if isinstance(dstT, tuple):
                        qte, qto = dstT
                        nc.scalar.activation(
                            out=qte[0:HD, :, tcol:tcol + P],
                            in_=tpw[0:HD], func=AF.Copy)
                        nc.scalar.activation(
                            out=qto[HD:P, :, tcol:tcol + P],
                            in_=tpw[HD:P], func=AF.Copy)
                    else:
                        nc.scalar.activation(
                            out=dstT[:, :, tcol:tcol + P], in_=tpw,
                            func=AF.Copy)

                # K/V tiles (full T, host-permuted so own q-tiles sit
                # at tt % 4 == 3); Q projected inline for those
                for tt in range(TTILES):
                    is_own = (tt % 4 == 3)
                    if tt in pref:
                        xt, ct, st = pref[tt]
                    else:
                        if is_own:
                            # own rows keep f32 x: it is the residual
                            xt = xq_sb[tt // 4]
                            nc.sync.dma_start(
                                out=xt,
                                in_=prm["xown"][(tt // 4) * P:
                                                (tt // 4 + 1) * P, :])
                        else:
                            xt = pa.tile([P, C], BF16, tag="xt",
                                         name="xt")
                            nc.sync.dma_start(
                                out=xt,
                                in_=prm["xbf"][tt * P:(tt + 1) * P, :])
                        ct = tabp.tile([P, 64], BF16, tag="cos",
                                       name="cos")
                        st = tabp.tile([P, 64], BF16, tag="sin",
                                       name="sin")
                        nc.sync.dma_start(
                            out=ct, in_=prm["cose"][tt * P:(tt + 1) * P, :])
                        nc.sync.dma_start(
                            out=st, in_=prm["sine"][tt * P:(tt + 1) * P, :])
                    f = norm_to(xt, hTm, tt * P, "n1")
                    # fold 1/rms into the rope tables (once per tile,
                    # shared by K and Q)
                    cts = tabp.tile([P, 64], BF16, tag="cts", name="cts")
                    sts = tabp.tile([P, 64], BF16, tag="sts", name="sts")
                    nc.vector.tensor_scalar_mul(cts, ct, f)
                    nc.vector.tensor_scalar_mul(sts, st, f)

                    def do_q():
                        qps = []
                        for fc in range(2):
                            pq = qkpp.tile([P, 512], F32, tag="pqkv",
                                           name="pqkv")
                            project(pq, hTm, tt * P, wq8, fc)
                            qps.append(pq)
                        rope_tp(qps, cts, sts, (QTmE, QTmO),
                                (tt // 4) * P, "q")

                    kps = []
                    for fc in range(2):
                        pk = qkpp.tile([P, 512], F32, tag="pqkv",
                                       name="pqkv")
                        project(pk, hTm, tt * P, wk8, fc)
                        kps.append(pk)
                    if tt == TTILES - 1:
                        # the last tile's Q chain gates all of attention;
                        # its K/V results are not needed until ~5us in
                        rope_tp(kps, cts, sts, KTm, tt * P, "k")
                        do_q()
                        for fc in range(2):
                            pv = qkpp.tile([P, 512], F32, tag="pqkv",
                                           name="pqkv")
                            project(pv, hTm, tt * P, wv8, fc)
                            nc.vector.tensor_scalar_mul(
                                Vm[:, tt, fc * 8:(fc + 1) * 8, 0:HD],
                                pv.rearrange("p (h d) -> p h d", d=HD),
                                f)
                        nc.gpsimd.memset(Vm[:, tt, :, HD:HD + 1], 1.0)
                    else:
                        # V (and Q) projections keep the PE busy while the
                        # DVE ropes K from its psum chunks
                        for fc in range(2):
                            pv = qkpp.tile([P, 512], F32, tag="pqkv",
                                           name="pqkv")
                            project(pv, hTm, tt * P, wv8, fc)
                            nc.vector.tensor_scalar_mul(
                                Vm[:, tt, fc * 8:(fc + 1) * 8, 0:HD],
                                pv.rearrange("p (h d) -> p h d", d=HD),
                                f)
                        nc.gpsimd.memset(Vm[:, tt, :, HD:HD + 1], 1.0)
                        rope_tp(kps, cts, sts, KTm, tt * P, "k")
                        if is_own:
                            do_q()

        if upto == "pa":
            _dummy_out()
            return

        # ---------------- Phase B: attention ----------------------------
        nc.gpsimd.dma_start(
            out=wpj, in_=prm["wproj"].rearrange("(cc p) o -> p cc o", p=P))
        with tc.tile_pool(name="pt", bufs=3) as ptp, \
             tc.tile_pool(name="rd", bufs=2) as rdp, \
             tc.tile_pool(name="stps", bufs=2, space="PSUM") as stpp, \
             tc.tile_pool(name="ytps", bufs=2, space="PSUM") as ytpp, \
             tc.tile_pool(name="rbps", bufs=1, space="PSUM") as rbpp:
            def make_tail(ytp, hc, hp):
                """Head tail, split: the DVE reciprocal is emitted
                IMMEDIATELY (its engine-count wait then covers only this
                head's PE stream); the PE broadcast + DVE normalize are
                deferred into the next head's score stream so the PE has
                queued work while the DVE reciprocal runs."""
                denr = rdp.tile([HD + 1, QROWS], BF16, tag="denr",
                                name="denr")
                # 1/D = exp(-ln D) on the scalar engine: the DVE
                # reciprocal on a (1,512) row costs 3.4us serial (one
                # lane); ln+exp stream in ~0.9us and share the phase-B
                # exp activation table (no table switch). denr is bf16
                # anyway, so table precision is not the limiter.
                dln = rdp.tile([1, QROWS], F32, tag="dln", name="dln")
                nc.scalar.activation(out=dln, in_=ytp[HD:HD + 1, :],
                                     func=AF.Ln)
                nc.scalar.activation(out=denr[HD:HD + 1, :], in_=dln,
                                     func=AF.Exp, scale=-1.0)

                def tail():
                    rdb = rbpp.tile([HD, QROWS], F32, tag="rdb",
                                    name="rdb")
                    nc.tensor.matmul(rdb, lhsT=ones_bf[HD:HD + 1, :],
                                     rhs=denr[HD:HD + 1, :],
                                     start=True, stop=True)
                    rdbs = rdp.tile([HD, QROWS], BF16, tag="rdbs",
                                    name="rdbs")
                    nc.vector.tensor_copy(rdbs, rdb)
                    if hp == 0:
                        nc.vector.tensor_tensor(
                            out=yT2m[0:HD, hc, :], in0=ytp[0:HD, :],
                            in1=rdbs, op=mybir.AluOpType.mult)
                    else:
                        yn = rdp.tile([HD, QROWS], F8, tag="yn",
                                      name="yn")
                        nc.vector.tensor_tensor(
                            out=yn, in0=ytp[0:HD, :], in1=rdbs,
                            op=mybir.AluOpType.mult)
                        nc.sync.dma_start(out=yT2m[HD:P, hc, :], in_=yn)
                return tail

            pending_tail = None
            pending_av = None
            for h in range(NH):
                hc = h // 2
                hp = (h % 2) * HD
                ytp = ytpp.tile([HD + 1, QROWS], F32, tag="ytp", name="ytp")
                for ktp in (0, 2, 4, 6, 8, 10, 12):
                    if ktp == 12 and pending_tail is not None:
                        pending_tail()
                        pending_tail = None
                    quad = ktp >= 12
                    nkt = 4 if quad else 2
                    qo = (ktp // 4) * P
                    w = QROWS - qo
                    if quad:
                        stp2 = stpp.tile([P, 4, w], F32,
                                         tag=f"stpq{ktp}",
                                         name=f"stpq{ktp}", bufs=1)
                        so = 0
                    else:
                        stp2 = stpp.tile([P, 2, 512], F32, tag="stp2",
                                         name="stp2")
                        so = qo
                    for i in range(nkt):
                        kt = ktp + i
                        nc.tensor.matmul(
                            stp2[:, i, so:],
                            lhsT=KTm[:, hc, kt * P:(kt + 1) * P],
                            rhs=(QTmE if hp == 0 else QTmO)[:, hc, qo:],
                            start=True, stop=True,
                        )
                    pt2 = ptp.tile([P, 4, 512], F8, tag="pt2",
                                   name="pt2")
                    # bias -2: exp(s) can reach ~675 > fp8e4m3 max 448
                    # (-> NaN); the constant shift cancels exactly
                    # between numerator and denominator
                    nc.scalar.activation(out=pt2[:, :nkt, qo:],
                                         in_=stp2[:, :, so:],
                                         func=AF.Exp, scale=EXPSCALE,
                                         bias=nshift)
                    # AVs of the PREVIOUS block run here, after this
                    # block's scores: by then the previous exp+mask have
                    # long finished, so the PE never waits (and keeps
                    # its p-state ramp)
                    if pending_av is not None:
                        pending_av()
                    nc.vector.tensor_mul(
                        pt2[:, :nkt, qo:qo + P],
                        pt2[:, :nkt, qo:qo + P],
                        smask[:, ktp:ktp + nkt, :])

                    def make_av(ktp=ktp, nkt=nkt, qo=qo, pt2=pt2,
                                ytp=ytp, h=h):
                        def av():
                            if ktp < 12:
                                # one full-width DR AV: the deferral
                                # guarantees the stripe mask finished,
                                # so masked + below-diagonal columns go
                                # in a single matmul
                                nc.tensor.matmul(
                                    ytp[:, qo:],
                                    lhsT=Vm[:, ktp:ktp + 2, h, :],
                                    rhs=pt2[:, 0:2, qo:],
                                    start=(ktp == 0), stop=False,
                                    skip_group_check=True, perf_mode=DR,
                                )
                                return
                            for ii in range(nkt // 2):
                                kt = ktp + 2 * ii
                                nc.tensor.matmul(
                                    ytp[:, qo:qo + P],
                                    lhsT=Vm[:, kt:kt + 2, h, :],
                                    rhs=pt2[:, 2 * ii:2 * ii + 2,
                                            qo:qo + P],
                                    start=False,
                                    stop=(kt + 1 == TTILES - 1),
                                    skip_group_check=True, perf_mode=DR,
                                )
                        return av

                    pending_av = make_av()
                pending_av()
                pending_av = None
                pending_tail = make_tail(ytp, hc, hp)
            pending_tail()

        if upto == "pb":
            _dummy_out()
            return

        # ---------------- Phase C: attn proj + residual + norm2 ---------
        x2p_pool = es.enter_context(tc.tile_pool(name="x2", bufs=1))
        x2sb = [x2p_pool.tile([P, C], F32, tag=f"x2_{q}", name=f"x2_{q}")
                for q in range(QTILES)]
        h2Tp = es.enter_context(tc.tile_pool(name="h2T", bufs=1))
        h2Tm = h2Tp.tile([P, CCH, QROWS], BF16, tag="h2Tm", name="h2Tm")
        with tc.tile_pool(name="pc", bufs=2) as pc, \
             tc.tile_pool(name="stat2", bufs=4) as stat2, \
             tc.tile_pool(name="x2ps", bufs=3, space="PSUM") as x2pp, \
             tc.tile_pool(name="tp2ps", bufs=1, space="PSUM") as tp2pp:
            scratch2 = pc.tile([P, C], F32, tag="sq2", name="sq2", bufs=1)

            def proj_qt(j):
                x2p = x2pp.tile([P, C], F32, tag="x2p", name="x2p")
                for half in range(2):
                    for cp in range(CCP):
                        nc.tensor.matmul(
                            x2p[:, half * 512:(half + 1) * 512],
                            lhsT=yT2m[:, 2 * cp:2 * cp + 2,
                                      j * P:(j + 1) * P],
                            rhs=wpj[:, 2 * cp:2 * cp + 2,
                                    half * 512:(half + 1) * 512],
                            start=(cp == 0), stop=(cp == CCP - 1),
                            perf_mode=DR,
                        )
                return x2p

            x2ps = [proj_qt(0), proj_qt(1)]
            for j in range(QTILES):
                if j + 2 < QTILES:
                    x2ps.append(proj_qt(j + 2))
                x2p = x2ps[j]
                # x2p holds 256*(y @ Wproj): fp8 carries 16y, wpj 16W
                x2s = pc.tile([P, C], F32, tag="x2s", name="x2s")
                nc.scalar.activation(out=x2s, in_=x2p, func=AF.Copy,
                                     scale=1.0 / 256)
                nc.vector.tensor_add(x2sb[j], x2s, xq_sb[j])
                ssq2 = stat2.tile([P, 1], F32, tag="ssq2", name="ssq2")
                nc.scalar.activation(out=scratch2, in_=x2sb[j],
                                     func=AF.Square, accum_out=ssq2)
                f2 = stat2.tile([P, 1], F32, tag="f2", name="f2")
                nc.scalar.activation(out=f2, in_=ssq2, func=AF.Sqrt,
                                     bias=eps_t, scale=1.0 / C)
                nc.vector.reciprocal(f2, f2)
                h2 = pc.tile([P, C], BF16, tag="h2", name="h2")
                nc.scalar.activation(out=h2, in_=x2sb[j], func=AF.Copy,
                                     scale=f2)
                tpw = tp2pp.tile([P, CCH, P], BF16, tag="tp2", name="tp2")
                for dc in range(CCH):
                    nc.tensor.transpose(
                        tpw[:, dc, :], h2[:, dc * P:(dc + 1) * P],
                        identity)
                nc.scalar.activation(
                    out=h2Tm[:, :, j * P:(j + 1) * P], in_=tpw,
                    func=AF.Copy)

        if upto == "pc":
            _dummy_out()
            return

        # ---------------- Phase D: SwiGLU -> mT --------------------------
        with tc.tile_pool(name="mt", bufs=1) as mtp, \
             tc.tile_pool(name="pew", bufs=4) as pew:
            mTm = mtp.tile([P, HSB, QROWS], BF16, tag="mTm", name="mTm")
            # pre-issue the first down-proj weight loads during phase D
            # so phase E's first matmuls fire at the boundary
            wdbs = {}
            for hs0 in range(3):
                wdb0 = pew.tile([P, C], BF16, tag="wdb", name="wdb")
                nc.gpsimd.dma_start(
                    out=wdb0, in_=prm["wd"][hs0 * P:(hs0 + 1) * P, :])
                wdbs[hs0] = wdb0
            with tc.tile_pool(name="pdw", bufs=3) as pdw, \
                 tc.tile_pool(name="pds", bufs=2) as pds, \
                 tc.tile_pool(name="abps", bufs=2, space="PSUM") as abpp:
                for hs in range(HSB):
                    w1b = pdw.tile([P, CCH, P], BF16, tag="w1b",
                                   name="w1b")
                    nc.sync.dma_start(out=w1b, in_=prm["w1"][hs])
                    w2b = pdw.tile([P, CCH, P], BF16, tag="w2b",
                                   name="w2b")
                    nc.sync.dma_start(out=w2b, in_=prm["w2"][hs])
                    ap_ = abpp.tile([P, QROWS], F32, tag="ap", name="ap")
                    bp_ = abpp.tile([P, QROWS], F32, tag="bp", name="bp")
                    # first block split by q-chunk so the MLP starts as
                    # soon as the first norm2 chain lands
                    qsl = ([slice(j * P, (j + 1) * P) for j in range(4)]
                           if hs <= 1 else [slice(0, QROWS)])
                    for sl in qsl:
                        for cc in range(CCH):
                            nc.tensor.matmul(
                                ap_[:, sl], lhsT=w1b[:, cc, :],
                                rhs=h2Tm[:, cc, sl],
                                start=(cc == 0), stop=(cc == CCH - 1))
                    for sl in qsl:
                        for cc in range(CCH):
                            nc.tensor.matmul(
                                bp_[:, sl], lhsT=w2b[:, cc, :],
                                rhs=h2Tm[:, cc, sl],
                                start=(cc == 0), stop=(cc == CCH - 1))
                    sT = pds.tile([P, QROWS], BF16, tag="sT", name="sT")
                    nc.scalar.activation(out=sT, in_=ap_, func=AF.Sigmoid)
                    nc.vector.tensor_tensor(
                        out=sT, in0=sT, in1=bp_, op=mybir.AluOpType.mult)
                    nc.vector.tensor_tensor(
                        out=mTm[:, hs, :], in0=sT, in1=ap_,
                        op=mybir.AluOpType.mult)

            if upto == "pd":
                _dummy_out()
                return
            # ---------------- Phase E: down proj + residual -------------
            with tc.tile_pool(name="peo", bufs=2) as peo, \
                 tc.tile_pool(name="x3ps", bufs=1, space="PSUM") as x3pp:
                x3p = [x3pp.tile([P, C], F32, tag=f"x3_{q}",
                                 name=f"x3_{q}") for q in range(QTILES)]
                for hs in range(HSB):
                    if hs in wdbs:
                        wdb = wdbs[hs]
                    else:
                        wdb = pew.tile([P, C], BF16, tag="wdb",
                                       name="wdb")
                        nc.gpsimd.dma_start(
                            out=wdb, in_=prm["wd"][hs * P:(hs + 1) * P, :])
                    for j in range(QTILES):
                        for half in range(2):
                            nc.tensor.matmul(
                                x3p[j][:, half * 512:(half + 1) * 512],
                                lhsT=mTm[:, hs, j * P:(j + 1) * P],
                                rhs=wdb[:, half * 512:(half + 1) * 512],
                                start=(hs == 0), stop=(hs == HSB - 1),
                            )
                for j in range(QTILES):
                    osb = peo.tile([P, C], F32, tag="osb", name="osb")
                    for half in range(2):
                        sl = slice(half * 512, (half + 1) * 512)
                        nc.vector.tensor_add(osb[:, sl], x3p[j][:, sl],
                                             x2sb[j][:, sl])
                        nc.sync.dma_start(
                            out=prm["out"][j * P:(j + 1) * P, sl],
                            in_=osb[:, sl])


def build_bass(upto="full", repeat=1):
    nc = bass.Bass("TRN2", target_bir_lowering=False, debug=False,
                   num_devices=8)
    prm = {}

    def inp(name, shape, dtype=F32):
        prm[name] = nc.declare_dram_parameter(name, list(shape), dtype,
                                              isOutput=False).ap()

    inp("xbf", (T, C), BF16)
    inp("xown", (QROWS, C))
    inp("cose", (T, 64), BF16)
    inp("sine", (T, 64), BF16)
    inp("smask", (TTILES, P, P), F8)
    inp("wq8", (CCH, P, C), F8)
    inp("wk8", (CCH, P, C), F8)
    inp("wv8", (CCH, P, C), F8)
    inp("wproj", (C, C), F8)
    inp("w1", (HSB, P, CCH, P), BF16)
    inp("w2", (HSB, P, CCH, P), BF16)
    inp("wd", (NHID, C), BF16)
    prm["out"] = nc.declare_dram_parameter("out", [QROWS, C], F32,
                                           isOutput=True).ap()
    with tile.TileContext(nc) as tc:
        for r in range(repeat):
            if r == repeat - 1:
                _emit(tc, nc, prm, upto=upto)
            else:
                sink = nc.dram_tensor(f"outsink{r}", [QROWS, C], F32).ap()
                _emit(tc, nc, dict(prm, out=sink), upto=upto)
    return nc


# ------------------------------------------------------------- host glue
def _rope_tables_expanded():
    """(T, 64) bf16 tables: col f = cos(t * theta_{f//2}); broadcast
    across the 16 heads on-chip (stride-0 AP)."""
    theta = (1.0 / (10000.0 ** (np.arange(0, HD, 2, dtype=np.float32)
                                / np.float32(HD)))).astype(np.float32)
    ang = np.outer(np.arange(T, dtype=np.float32), theta)  # (T, 32)
    cos = np.cos(ang).astype(np.float32)
    sin = np.sin(ang).astype(np.float32)
    cose = np.repeat(cos, 2, axis=1).astype(ml_dtypes.bfloat16)
    sine = np.repeat(sin, 2, axis=1).astype(ml_dtypes.bfloat16)
    return cose, sine


def _to_f8(w):
    return np.clip(w * WS, -240.0, 240.0).astype(ml_dtypes.float8_e4m3)


def _dr_layout(w):
    """(C, 1024) -> (CCH, 128, 1024)."""
    return np.ascontiguousarray(w.reshape(CCH, P, w.shape[1]))


def core_rows(c):
    t = c % 4
    tiles = [t, t + 4, t + 8, t + 12]
    return np.concatenate([np.arange(a * P, (a + 1) * P) for a in tiles])


def make_in_maps(x, y_mask, Wqkv, Wattn_proj, scale1, scale2, Wfc1, Wfc2,
                 Wmlp_proj):
    f = np.float32
    bf = ml_dtypes.bfloat16
    Wq = (scale1[:, None] * Wqkv[:, 0:C]).astype(f)
    Wk = (scale1[:, None] * Wqkv[:, C:2 * C]).astype(f)
    Wv = (scale1[:, None] * Wqkv[:, 2 * C:3 * C]).astype(f)
    wq8 = _dr_layout(_to_f8(Wq))
    wk8 = _dr_layout(_to_f8(Wk))
    wv8 = _dr_layout(_to_f8(Wv))
    wproj = _to_f8(Wattn_proj.astype(f))
    W1f = (scale2[:, None] * Wfc1).astype(f)
    W2f = (scale2[:, None] * Wfc2).astype(f)
    # (C, NHID) -> (HSB, P, CCH, P): w1[hs][p][cc][j] = W[cc*128+p, hs*128+j]
    w1 = np.ascontiguousarray(
        W1f.reshape(CCH, P, HSB, P).transpose(2, 1, 0, 3)).astype(bf)
    w2 = np.ascontiguousarray(
        W2f.reshape(CCH, P, HSB, P).transpose(2, 1, 0, 3)).astype(bf)
    wd = np.ascontiguousarray(Wmlp_proj.astype(f)).astype(bf)
    cose, sine = _rope_tables_expanded()

    kidx = np.arange(T)
    in_maps = []
    for c in range(8):
        b = c // 4
        c4 = c % 4
        # permute tiles: within each group of 4, own tile goes last
        tl = []
        for g in range(4):
            tl += [4 * g + r for r in range(4) if r != c4]
            tl.append(4 * g + c4)
        prows = np.concatenate(
            [np.arange(t * P, (t + 1) * P) for t in tl])
        ym = np.zeros(T, bool)
        ym[:64] = y_mask[b].astype(bool)
        # stripe masks: for k-tile kt (original tile tl[kt]), q-tile
        # j = kt//4 (original own tile c4+4j), 0/1 allowed
        smask = np.zeros((TTILES, P, P), np.float32)
        for kt in range(TTILES):
            j = kt // 4
            qabs = kidx[(c4 + 4 * j) * P:(c4 + 4 * j + 1) * P]
            kabs = kidx[tl[kt] * P:(tl[kt] + 1) * P]
            allowed = (kabs[:, None] <= qabs[None, :]) | (
                ym[kabs][:, None] & ym[qabs][None, :])
            smask[kt] = allowed.astype(np.float32)
        xperm = x[b][prows].astype(f)
        own_rows = np.concatenate(
            [np.arange((4 * j + 3) * P, (4 * j + 4) * P)
             for j in range(4)])
        in_maps.append({
            "xbf": np.ascontiguousarray(
                xperm.astype(ml_dtypes.bfloat16)),
            "xown": np.ascontiguousarray(xperm[own_rows]),
            "cose": np.ascontiguousarray(cose[prows]),
            "sine": np.ascontiguousarray(sine[prows]),
            "smask": smask.astype(ml_dtypes.float8_e4m3),
            "wq8": wq8, "wk8": wk8, "wv8": wv8,
            "wproj": wproj, "w1": w1, "w2": w2, "wd": wd,
        })
    return in_maps


_NC_CACHE = None


def kernel(**inputs):
    global _NC_CACHE
    in_maps = make_in_maps(**{k: np.asarray(v) for k, v in inputs.items()})
    if _NC_CACHE is None:
        _NC_CACHE = build_bass()
    res = run_bass_kernel_spmd(_NC_CACHE, in_maps, core_ids=list(range(8)))
    out = np.empty((B, T, C), np.float32)
    for c in range(8):
        out[c // 4, core_rows(c)] = res.results[c]["out"]
    return out



# revision 80
# speedup vs baseline: 1.0016x; 1.0016x over previous
"""Trainium2 Bass kernel v3 for the dense transformer block.

Distribution (zero-collective): data-parallel over batch (B=2) x
query-row-parallel over token tiles (4 ways) => 8 cores. Each core
computes K/V (all heads, full T) and Q (own 512 rows, gathered
round-robin) and produces the final output for its 4 q-row tiles.

One program for all 8 cores: per-core differences (which q rows, the
causal/text stripe masks) are carried by input data. With q-tile j of
core c at absolute tile (c%4)+4j, k-tile kt is strictly above the
diagonal for j < kt//4 (skipped), fully allowed for j > kt//4, and the
single stripe j == kt//4 gets a data-driven 0/1 mask multiplied in
after exp.

Precision: fp8e4 + DoubleRow (2x PE) for the QKV projections, the
attention AV product (exp probs + V in fp8, exp shifted by -2 so it
cannot overflow fp8's 448 max) and the attention out-projection
(descaled 1/256 into the residual); bf16 for scores and the MLP (fp8
there fails the 2e-2 gate); fp32 PSUM accumulation and residuals.

Phase A: x is loaded bf16 for non-own tiles (norm-only use) and f32
for own rows (residual); the 1/rms norm scale is folded into the rope
tables and the V psum copy so the PE transposes depend only on the x
DMA. Rope tables are compact (T,64) and stride-0-broadcast across
heads. RoPE combines run on GpSimd, mults on DVE from PSUM.

Phase B: per-head softmax denominator 1/D computed as exp(-ln D) on
the scalar engine (the DVE reciprocal on a (1,512) row costs 3.4us on
one lane and stalled the PE); the PE denominator-broadcast + normalize
of head h are deferred into head h+1's score stream, and each block's
AV matmuls are deferred one ktp block behind its scores.

DMA: x/tables on the sync queue, K/V weight halves split across the
scalar+gpsimd queues, MLP weights w1/sync w2/sync wd/gpsimd with the
first down-proj blocks prefetched during phase D. No DRAM scratch.
"""

import numpy as np
import ml_dtypes

import concourse.bass as bass
import concourse.mybir as mybir
import concourse.tile as tile
from concourse.bass_utils import run_bass_kernel_spmd
from concourse.masks import make_identity
from concourse.vector_clock import ScopedClock

# ---------------------------------------------------------------- shapes
B = 2
T = 2048
C = 1024
NH = 16
HD = 64
NHID = 2816
EPS = 1e-5
P = 128
TTILES = T // P      # 16
QTILES = 4           # q-row tiles per core
QROWS = QTILES * P   # 512
CCH = C // P         # 8 contraction chunks
CCP = CCH // 2       # 4 DoubleRow pairs
HSB = NHID // P      # 22 hidden blocks
F32 = mybir.dt.float32
BF16 = mybir.dt.bfloat16
F8 = mybir.dt.float8e4
WS = 16.0            # fp8 weight scale
EXPSCALE = 0.125 / (WS * WS)

AF = mybir.ActivationFunctionType
DR = mybir.MatmulPerfMode.DoubleRow

# ------------------------------------------------- TileContext drain patch
_DRAIN_CAP = 1


def _patched_drain_and_barrier(self, tick_clock, wait_clock):
    nc = self.nc
    drain_inst = nc.sync.drain()
    wait_clock.add_sem_waits(
        drain_inst.ins, ScopedClock({None: tick_clock.global_clock})
    )
    waits = list(drain_inst.ins.sync_info.on_wait)
    if len(waits) > _DRAIN_CAP:
        upd = list(drain_inst.ins.sync_info.on_update)
        drain_inst.ins.sync_info = mybir.SyncInfo(
            on_wait=waits[:_DRAIN_CAP], on_update=upd
        )
        rest = waits[_DRAIN_CAP:]
        while rest:
            d2 = nc.sync.drain()
            d2.ins.sync_info = mybir.SyncInfo(
                on_wait=rest[:_DRAIN_CAP], on_update=[]
            )
            rest = rest[_DRAIN_CAP:]
    nc.all_engine_barrier()
    popped = nc._tile_sem_poison_stack.pop()
    assert popped is self._sem_poison
    nc.clear_and_free_semaphores(list(self.sems.allocated().values()))
    nc.all_engine_barrier()


tile.TileContext._drain_and_barrier = _patched_drain_and_barrier

_JSON_WAIT_CAP = 1
_WAIT_CAPS = {}
_WAIT_CAP_DEFAULT = 1


def _split_bir_waits(bir_bytes):
    import orjson
    d = orjson.loads(bir_bytes)
    n_split = 0
    for fn in d["functions"]:
        for blk in fn["blocks"]:
            out = []
            for inst in blk["instructions"]:
                si = inst.get("sync_info")
                waits = (si or {}).get("on_wait") or []
                cap = _WAIT_CAPS.get(inst.get("opcode"), _WAIT_CAP_DEFAULT)
                if len(waits) > cap:
                    keep = waits[-cap:]
                    extra = waits[:-cap]
                    w_i = 0
                    while extra:
                        chunk = extra[:_JSON_WAIT_CAP]
                        extra = extra[_JSON_WAIT_CAP:]
                        nop = {
                            "name": f"{inst['name']}_w{w_i}",
                            "opcode": "NoOp",
                            "engine": inst["engine"],
                            "ins": [],
                            "outs": [],
                            "sync_info": {"on_wait": chunk, "on_update": []},
                        }
                        if "debug" in inst:
                            nop["debug"] = inst["debug"]
                        out.append(nop)
                        w_i += 1
                    si["on_wait"] = keep
                    n_split += 1
                out.append(inst)
            blk["instructions"] = out
    return orjson.dumps(d), n_split


import concourse.bass_utils as _bass_utils_mod
import concourse.bass2jax as _bass2jax_mod

_orig_compile_bir_kernel = _bass_utils_mod.compile_bir_kernel


def _patched_compile_bir_kernel(ant_bir_str, compile_dir_path, **kwargs):
    fixed, n = _split_bir_waits(ant_bir_str)
    return _orig_compile_bir_kernel(fixed, compile_dir_path, **kwargs)


_bass_utils_mod.compile_bir_kernel = _patched_compile_bir_kernel
_bass2jax_mod.compile_bir_kernel = _patched_compile_bir_kernel




# ------------------------------------------------------------ device code
def _emit(tc, nc, prm, upto="full"):
    from contextlib import ExitStack

    def _dummy_out():
        nc.sync.dma_start(out=prm["out"][:, :], in_=prm["xown"][:, :])

    with ExitStack() as es:
        constp = es.enter_context(tc.tile_pool(name="const", bufs=1))
        identity_f = constp.tile([P, P], F32, tag="idf", name="idf")
        make_identity(nc, identity_f)
        identity = constp.tile([P, P], BF16, tag="idb", name="idb")
        nc.vector.tensor_copy(identity, identity_f)
        eps_t = constp.tile([P, 1], F32, tag="eps", name="eps")
        nc.vector.memset(eps_t, EPS)
        nshift = constp.tile([P, 1], F32, tag="nshift", name="nshift")
        nc.vector.memset(nshift, -2.0)
        ones_bf = constp.tile([P, HD], BF16, tag="ones", name="ones")
        nc.vector.memset(ones_bf, 1.0)
        smask = constp.tile([P, TTILES, P], F8, tag="smask", name="smask")

        yT2p = es.enter_context(tc.tile_pool(name="yT2", bufs=1))
        yT2m = yT2p.tile([P, CCH, QROWS], F8, tag="yT2m", name="yT2m")
        xqp = es.enter_context(tc.tile_pool(name="xq", bufs=1))
        xq_sb = [xqp.tile([P, C], F32, tag=f"xq{j}", name=f"xq{j}")
                 for j in range(QTILES)]
        wpjp = es.enter_context(tc.tile_pool(name="wpj", bufs=1))
        wpj = wpjp.tile([P, CCH, C], F8, tag="wpj", name="wpj")
        # attention SBUF arrays: kept alive to the end (SBUF fits) so
        # the B->C transition needs no drain+barrier pair
        attnp = es.enter_context(tc.tile_pool(name="attnbig", bufs=1))
        KTm = attnp.tile([P, CCH, T], BF16, tag="KTm", name="KTm")
        # Q in two head-parity copies, the other head's partitions
        # zeroed: scores then use full-128-partition operands (64-high
        # matmuls stream at ~half the PE column rate)
        QTmE = attnp.tile([P, CCH, QROWS], BF16, tag="QTmE", name="QTmE")
        QTmO = attnp.tile([P, CCH, QROWS], BF16, tag="QTmO", name="QTmO")
        nc.gpsimd.memset(QTmE, 0.0)
        nc.gpsimd.memset(QTmO, 0.0)
        Vm = attnp.tile([P, TTILES, NH, HD + 1], F8, tag="Vm", name="Vm")
        hTm = attnp.tile([P, CCH, T], F8, tag="hTm", name="hTm")

        # ---------------- Phase A: norm + QKV + rope + transposes --------
        if True:
            with tc.tile_pool(name="w8", bufs=1) as w8p, \
                 tc.tile_pool(name="pa", bufs=3) as pa, \
                 tc.tile_pool(name="stat", bufs=6) as statp, \
                 tc.tile_pool(name="rope", bufs=4) as ropep, \
                 tc.tile_pool(name="tab", bufs=3) as tabp, \
                 tc.tile_pool(name="qkps", bufs=4, space="PSUM") as qkpp, \
                 tc.tile_pool(name="tpps", bufs=2, space="PSUM") as tppp:
                # startup order: tile-0 data, K weights, more tiles, V/Q
                pref = {}

                def pref_tile(tt):
                    # non-own tiles feed only rmsnorm -> K/V: bf16 x is
                    # plenty (fp8 QKV quantization dominates)
                    xt = pa.tile([P, C], BF16, tag="xt", name="xt")
                    nc.sync.dma_start(
                        out=xt, in_=prm["xbf"][tt * P:(tt + 1) * P, :])
                    ct = tabp.tile([P, 64], BF16, tag="cos", name="cos")
                    st = tabp.tile([P, 64], BF16, tag="sin", name="sin")
                    nc.sync.dma_start(
                        out=ct, in_=prm["cose"][tt * P:(tt + 1) * P, :])
                    nc.sync.dma_start(
                        out=st, in_=prm["sine"][tt * P:(tt + 1) * P, :])
                    pref[tt] = (xt, ct, st)

                pref_tile(0)
                wq8 = w8p.tile([P, CCH, C], F8, tag="wq8", name="wq8")
                wk8 = w8p.tile([P, CCH, C], F8, tag="wk8", name="wk8")
                wv8 = w8p.tile([P, CCH, C], F8, tag="wv8", name="wv8")
                # split K/V weight loads by fc half across idle queues so
                # the first projections start ~4us earlier
                nc.scalar.dma_start(
                    out=wk8[:, :, 0:512],
                    in_=prm["wk8"][:, :, 0:512].rearrange(
                        "cc p o -> p cc o"))
                nc.gpsimd.dma_start(
                    out=wk8[:, :, 512:1024],
                    in_=prm["wk8"][:, :, 512:1024].rearrange(
                        "cc p o -> p cc o"))
                nc.scalar.dma_start(
                    out=wv8[:, :, 0:512],
                    in_=prm["wv8"][:, :, 0:512].rearrange(
                        "cc p o -> p cc o"))
                nc.gpsimd.dma_start(
                    out=wv8[:, :, 512:1024],
                    in_=prm["wv8"][:, :, 512:1024].rearrange(
                        "cc p o -> p cc o"))
                nc.gpsimd.dma_start(
                    out=wq8, in_=prm["wq8"].rearrange("cc p o -> p cc o"))
                nc.gpsimd.dma_start(
                    out=smask, in_=prm["smask"].rearrange("kt p q -> p kt q"))
                pref_tile(1)
                pref_tile(2)
                scratch = pa.tile([P, C], F32, tag="sq_scr", name="sq_scr",
                                  bufs=1)

                def norm_to(xt, dstT, tcol, tag):
                    """Norm stats + transpose of RAW x into fp8
                    dstT[:, :, tcol:tcol+128]. The 1/rms scale is NOT
                    applied here: it folds into the rope tables (K/Q)
                    and the V psum copy, so the PE transposes depend
                    only on the x DMA, not the scalar norm chain.
                    Returns the (128,1) 1/rms tile."""
                    ssq = statp.tile([P, 1], F32, tag=f"{tag}ssq",
                                     name=f"{tag}ssq")
                    nc.scalar.activation(out=scratch, in_=xt,
                                         func=AF.Square, accum_out=ssq)
                    f = statp.tile([P, 1], F32, tag=f"{tag}f",
                                   name=f"{tag}f")
                    nc.scalar.activation(out=f, in_=ssq, func=AF.Sqrt,
                                         bias=eps_t, scale=1.0 / C)
                    nc.vector.reciprocal(f, f)
                    if xt.dtype != BF16:
                        xb = pa.tile([P, C], BF16, tag="xb", name="xb")
                        nc.gpsimd.tensor_copy(xb, xt)
                        src = xb
                    else:
                        src = xt
                    tpw = tppp.tile([P, CCH, P], BF16, tag="tpwh",
                                    name="tpwh")
                    for dc in range(CCH):
                        nc.tensor.transpose(
                            tpw[:, dc, :], src[:, dc * P:(dc + 1) * P],
                            identity)
                    nc.scalar.activation(
                        out=dstT[:, :, tcol:tcol + P], in_=tpw,
                        func=AF.Copy)
                    return f

                def project(dst_psum, srcT, scol, wsb, fc):
                    """QKV matmuls: (128 t-rows at srcT col scol) x
                    (feature chunk fc) into dst_psum (128, 512).
                    fp8 DoubleRow: two contraction chunks per pass."""
                    for cp in range(CCP):
                        nc.tensor.matmul(
                            dst_psum,
                            lhsT=srcT[:, 2 * cp:2 * cp + 2, scol:scol + P],
                            rhs=wsb[:, 2 * cp:2 * cp + 2,
                                    fc * 512:(fc + 1) * 512],
                            start=(cp == 0), stop=(cp == CCP - 1),
                            perf_mode=DR,
                        )

                def rope_tp(psums, ct, st, dstT, tcol, tag):
                    """RoPE from two (128,512) psum chunks + transpose into
                    dstT[:, :, tcol:tcol+128] (bf16). Split by column half
                    across DVE and GpSimd (GpSimd idles otherwise and the
                    rope chain is the phase-A critical path)."""
                    A = ropep.tile([P, C], BF16, tag="rA", name=f"{tag}A")
                    Bt = ropep.tile([P, C], BF16, tag="rB", name=f"{tag}B")
                    # compact (128,64) tables, stride-0-broadcast across
                    # the 8 heads of each 512-col psum chunk
                    ctb = ct.unsqueeze(1).to_broadcast([P, 8, 64])
                    stb = st.unsqueeze(1).to_broadcast([P, 8, 64])
                    A4 = A.rearrange("p (r f) -> p r f", f=64)
                    B4 = Bt.rearrange("p (r f) -> p r f", f=64)
                    for fc in range(2):
                        hsl = slice(fc * 8, (fc + 1) * 8)
                        ps4 = psums[fc].rearrange("p (r f) -> p r f", f=64)
                        nc.vector.tensor_mul(A4[:, hsl, :], ps4, ctb)
                        nc.vector.tensor_mul(B4[:, hsl, :], ps4, stb)
                    kr = ropep.tile([P, C], BF16, tag="rr", name=f"{tag}r")
                    A3 = A.rearrange("p (i two) -> p i two", two=2)
                    B3 = Bt.rearrange("p (i two) -> p i two", two=2)
                    k3 = kr.rearrange("p (i two) -> p i two", two=2)
                    # combines are SBUF-only: run them on GpSimd (PSUM is
                    # not GpSimd-accessible) to shorten the DVE chain
                    nc.gpsimd.tensor_sub(k3[:, :, 0], A3[:, :, 0],
                                         B3[:, :, 1])
                    nc.gpsimd.tensor_add(k3[:, :, 1], A3[:, :, 1],
                                         B3[:, :, 0])
                    tpw = tppp.tile([P, CCH, P], BF16, tag="tpw",
                                    name="tpw")
                    for dc in range(CCH):
                        nc.tensor.transpose(
                            tpw[:, dc, :], kr[:, dc * P:(dc + 1) * P],
                            identity)
                    if isinstance(dstT, tuple):
                        qte, qto = dstT
                        nc.scalar.activation(
                            out=qte[0:HD, :, tcol:tcol + P],
                            in_=tpw[0:HD], func=AF.Copy)
                        nc.scalar.activation(
                            out=qto[HD:P, :, tcol:tcol + P],
                            in_=tpw[HD:P], func=AF.Copy)
                    else:
                        nc.scalar.activation(
                            out=dstT[:, :, tcol:tcol + P], in_=tpw,
                            func=AF.Copy)

                # K/V tiles (full T, host-permuted so own q-tiles sit
                # at tt % 4 == 3); Q projected inline for those
                for tt in range(TTILES):
                    is_own = (tt % 4 == 3)
                    if tt in pref:
                        xt, ct, st = pref[tt]
                    else:
                        if is_own:
                            # own rows keep f32 x: it is the residual
                            xt = xq_sb[tt // 4]
                            nc.sync.dma_start(
                                out=xt,
                                in_=prm["xown"][(tt // 4) * P:
                                                (tt // 4 + 1) * P, :])
                        else:
                            xt = pa.tile([P, C], BF16, tag="xt",
                                         name="xt")
                            nc.sync.dma_start(
                                out=xt,
                                in_=prm["xbf"][tt * P:(tt + 1) * P, :])
                        ct = tabp.tile([P, 64], BF16, tag="cos",
                                       name="cos")
                        st = tabp.tile([P, 64], BF16, tag="sin",
                                       name="sin")
                        nc.sync.dma_start(
                            out=ct, in_=prm["cose"][tt * P:(tt + 1) * P, :])
                        nc.sync.dma_start(
                            out=st, in_=prm["sine"][tt * P:(tt + 1) * P, :])
                    f = norm_to(xt, hTm, tt * P, "n1")
                    # fold 1/rms into the rope tables (once per tile,
                    # shared by K and Q)
                    cts = tabp.tile([P, 64], BF16, tag="cts", name="cts")
                    sts = tabp.tile([P, 64], BF16, tag="sts", name="sts")
                    nc.vector.tensor_scalar_mul(cts, ct, f)
                    nc.vector.tensor_scalar_mul(sts, st, f)

                    def do_q():
                        qps = []
                        for fc in range(2):
                            pq = qkpp.tile([P, 512], F32, tag="pqkv",
                                           name="pqkv")
                            project(pq, hTm, tt * P, wq8, fc)
                            qps.append(pq)
                        rope_tp(qps, cts, sts, (QTmE, QTmO),
                                (tt // 4) * P, "q")

                    kps = []
                    for fc in range(2):
                        pk = qkpp.tile([P, 512], F32, tag="pqkv",
                                       name="pqkv")
                        project(pk, hTm, tt * P, wk8, fc)
                        kps.append(pk)
                    if tt == TTILES - 1:
                        # the last tile's Q chain gates all of attention;
                        # its K/V results are not needed until ~5us in
                        rope_tp(kps, cts, sts, KTm, tt * P, "k")
                        do_q()
                        for fc in range(2):
                            pv = qkpp.tile([P, 512], F32, tag="pqkv",
                                           name="pqkv")
                            project(pv, hTm, tt * P, wv8, fc)
                            nc.vector.tensor_scalar_mul(
                                Vm[:, tt, fc * 8:(fc + 1) * 8, 0:HD],
                                pv.rearrange("p (h d) -> p h d", d=HD),
                                f)
                        nc.gpsimd.memset(Vm[:, tt, :, HD:HD + 1], 1.0)
                    else:
                        # V (and Q) projections keep the PE busy while the
                        # DVE ropes K from its psum chunks
                        for fc in range(2):
                            pv = qkpp.tile([P, 512], F32, tag="pqkv",
                                           name="pqkv")
                            project(pv, hTm, tt * P, wv8, fc)
                            nc.vector.tensor_scalar_mul(
                                Vm[:, tt, fc * 8:(fc + 1) * 8, 0:HD],
                                pv.rearrange("p (h d) -> p h d", d=HD),
                                f)
                        nc.gpsimd.memset(Vm[:, tt, :, HD:HD + 1], 1.0)
                        rope_tp(kps, cts, sts, KTm, tt * P, "k")
                        if is_own:
                            do_q()

        if upto == "pa":
            _dummy_out()
            return

        # ---------------- Phase B: attention ----------------------------
        nc.gpsimd.dma_start(
            out=wpj, in_=prm["wproj"].rearrange("(cc p) o -> p cc o", p=P))
        with tc.tile_pool(name="pt", bufs=3) as ptp, \
             tc.tile_pool(name="rd", bufs=2) as rdp, \
             tc.tile_pool(name="stps", bufs=2, space="PSUM") as stpp, \
             tc.tile_pool(name="ytps", bufs=2, space="PSUM") as ytpp, \
             tc.tile_pool(name="rbps", bufs=1, space="PSUM") as rbpp:
            def make_tail(ytp, hc, hp):
                """Head tail, split: the DVE reciprocal is emitted
                IMMEDIATELY (its engine-count wait then covers only this
                head's PE stream); the PE broadcast + DVE normalize are
                deferred into the next head's score stream so the PE has
                queued work while the DVE reciprocal runs."""
                denr = rdp.tile([HD + 1, QROWS], BF16, tag="denr",
                                name="denr")
                # 1/D = exp(-ln D) on the scalar engine: the DVE
                # reciprocal on a (1,512) row costs 3.4us serial (one
                # lane); ln+exp stream in ~0.9us and share the phase-B
                # exp activation table (no table switch). denr is bf16
                # anyway, so table precision is not the limiter.
                dln = rdp.tile([1, QROWS], F32, tag="dln", name="dln")
                nc.scalar.activation(out=dln, in_=ytp[HD:HD + 1, :],
                                     func=AF.Ln)
                nc.scalar.activation(out=denr[HD:HD + 1, :], in_=dln,
                                     func=AF.Exp, scale=-1.0)

                def tail():
                    rdb = rbpp.tile([HD, QROWS], F32, tag="rdb",
                                    name="rdb")
                    nc.tensor.matmul(rdb, lhsT=ones_bf[HD:HD + 1, :],
                                     rhs=denr[HD:HD + 1, :],
                                     start=True, stop=True)
                    rdbs = rdp.tile([HD, QROWS], BF16, tag="rdbs",
                                    name="rdbs")
                    nc.vector.tensor_copy(rdbs, rdb)
                    if hp == 0:
                        nc.vector.tensor_tensor(
                            out=yT2m[0:HD, hc, :], in0=ytp[0:HD, :],
                            in1=rdbs, op=mybir.AluOpType.mult)
                    else:
                        yn = rdp.tile([HD, QROWS], F8, tag="yn",
                                      name="yn")
                        nc.vector.tensor_tensor(
                            out=yn, in0=ytp[0:HD, :], in1=rdbs,
                            op=mybir.AluOpType.mult)
                        nc.sync.dma_start(out=yT2m[HD:P, hc, :], in_=yn)
                return tail

            pending_tail = None
            pending_av = None
            for h in range(NH):
                hc = h // 2
                hp = (h % 2) * HD
                ytp = ytpp.tile([HD + 1, QROWS], F32, tag="ytp", name="ytp")
                for ktp in (0, 2, 4, 6, 8, 10, 12):
                    if ktp == 12 and pending_tail is not None:
                        pending_tail()
                        pending_tail = None
                    quad = ktp >= 12
                    nkt = 4 if quad else 2
                    qo = (ktp // 4) * P
                    w = QROWS - qo
                    if quad:
                        stp2 = stpp.tile([P, 4, w], F32,
                                         tag=f"stpq{ktp}",
                                         name=f"stpq{ktp}", bufs=1)
                        so = 0
                    else:
                        stp2 = stpp.tile([P, 2, 512], F32, tag="stp2",
                                         name="stp2")
                        so = qo
                    for i in range(nkt):
                        kt = ktp + i
                        nc.tensor.matmul(
                            stp2[:, i, so:],
                            lhsT=KTm[:, hc, kt * P:(kt + 1) * P],
                            rhs=(QTmE if hp == 0 else QTmO)[:, hc, qo:],
                            start=True, stop=True,
                        )
                    pt2 = ptp.tile([P, 4, 512], F8, tag="pt2",
                                   name="pt2")
                    # bias -2: exp(s) can reach ~675 > fp8e4m3 max 448
                    # (-> NaN); the constant shift cancels exactly
                    # between numerator and denominator
                    nc.scalar.activation(out=pt2[:, :nkt, qo:],
                                         in_=stp2[:, :, so:],
                                         func=AF.Exp, scale=EXPSCALE,
                                         bias=nshift)
                    # AVs of the PREVIOUS block run here, after this
                    # block's scores: by then the previous exp+mask have
                    # long finished, so the PE never waits (and keeps
                    # its p-state ramp)
                    if pending_av is not None:
                        pending_av()
                    nc.vector.tensor_mul(
                        pt2[:, :nkt, qo:qo + P],
                        pt2[:, :nkt, qo:qo + P],
                        smask[:, ktp:ktp + nkt, :])

                    def make_av(ktp=ktp, nkt=nkt, qo=qo, pt2=pt2,
                                ytp=ytp, h=h):
                        def av():
                            if ktp < 12:
                                # one full-width DR AV: the deferral
                                # guarantees the stripe mask finished,
                                # so masked + below-diagonal columns go
                                # in a single matmul
                                nc.tensor.matmul(
                                    ytp[:, qo:],
                                    lhsT=Vm[:, ktp:ktp + 2, h, :],
                                    rhs=pt2[:, 0:2, qo:],
                                    start=(ktp == 0), stop=False,
                                    skip_group_check=True, perf_mode=DR,
                                )
                                return
                            for ii in range(nkt // 2):
                                kt = ktp + 2 * ii
                                nc.tensor.matmul(
                                    ytp[:, qo:qo + P],
                                    lhsT=Vm[:, kt:kt + 2, h, :],
                                    rhs=pt2[:, 2 * ii:2 * ii + 2,
                                            qo:qo + P],
                                    start=False,
                                    stop=(kt + 1 == TTILES - 1),
                                    skip_group_check=True, perf_mode=DR,
                                )
                        return av

                    pending_av = make_av()
                pending_av()
                pending_av = None
                pending_tail = make_tail(ytp, hc, hp)
            pending_tail()

        if upto == "pb":
            _dummy_out()
            return

        # ---------------- Phase C: attn proj + residual + norm2 ---------
        x2p_pool = es.enter_context(tc.tile_pool(name="x2", bufs=1))
        x2sb = [x2p_pool.tile([P, C], F32, tag=f"x2_{q}", name=f"x2_{q}")
                for q in range(QTILES)]
        h2Tp = es.enter_context(tc.tile_pool(name="h2T", bufs=1))
        h2Tm = h2Tp.tile([P, CCH, QROWS], BF16, tag="h2Tm", name="h2Tm")
        with tc.tile_pool(name="pc", bufs=2) as pc, \
             tc.tile_pool(name="stat2", bufs=4) as stat2, \
             tc.tile_pool(name="x2ps", bufs=3, space="PSUM") as x2pp, \
             tc.tile_pool(name="tp2ps", bufs=1, space="PSUM") as tp2pp:
            scratch2 = pc.tile([P, C], F32, tag="sq2", name="sq2", bufs=1)

            def proj_qt(j):
                x2p = x2pp.tile([P, C], F32, tag="x2p", name="x2p")
                for half in range(2):
                    for cp in range(CCP):
                        nc.tensor.matmul(
                            x2p[:, half * 512:(half + 1) * 512],
                            lhsT=yT2m[:, 2 * cp:2 * cp + 2,
                                      j * P:(j + 1) * P],
                            rhs=wpj[:, 2 * cp:2 * cp + 2,
                                    half * 512:(half + 1) * 512],
                            start=(cp == 0), stop=(cp == CCP - 1),
                            perf_mode=DR,
                        )
                return x2p

            x2ps = [proj_qt(0), proj_qt(1)]
            for j in range(QTILES):
                if j + 2 < QTILES:
                    x2ps.append(proj_qt(j + 2))
                x2p = x2ps[j]
                # x2p holds 256*(y @ Wproj): fp8 carries 16y, wpj 16W
                x2s = pc.tile([P, C], F32, tag="x2s", name="x2s")
                nc.scalar.activation(out=x2s, in_=x2p, func=AF.Copy,
                                     scale=1.0 / 256)
                nc.vector.tensor_add(x2sb[j], x2s, xq_sb[j])
                ssq2 = stat2.tile([P, 1], F32, tag="ssq2", name="ssq2")
                nc.scalar.activation(out=scratch2, in_=x2sb[j],
                                     func=AF.Square, accum_out=ssq2)
                f2 = stat2.tile([P, 1], F32, tag="f2", name="f2")
                nc.scalar.activation(out=f2, in_=ssq2, func=AF.Sqrt,
                                     bias=eps_t, scale=1.0 / C)
                nc.vector.reciprocal(f2, f2)
                h2 = pc.tile([P, C], BF16, tag="h2", name="h2")
                nc.scalar.activation(out=h2, in_=x2sb[j], func=AF.Copy,
                                     scale=f2)
                tpw = tp2pp.tile([P, CCH, P], BF16, tag="tp2", name="tp2")
                for dc in range(CCH):
                    nc.tensor.transpose(
                        tpw[:, dc, :], h2[:, dc * P:(dc + 1) * P],
                        identity)
                nc.scalar.activation(
                    out=h2Tm[:, :, j * P:(j + 1) * P], in_=tpw,
                    func=AF.Copy)

        if upto == "pc":
            _dummy_out()
            return

        # ---------------- Phase D: SwiGLU -> mT --------------------------
        with tc.tile_pool(name="mt", bufs=1) as mtp, \
             tc.tile_pool(name="pew", bufs=4) as pew:
            mTm = mtp.tile([P, HSB, QROWS], BF16, tag="mTm", name="mTm")
            # pre-issue the first down-proj weight loads during phase D
            # so phase E's first matmuls fire at the boundary
            wdbs = {}
            for hs0 in range(3):
                wdb0 = pew.tile([P, C], BF16, tag="wdb", name="wdb")
                nc.gpsimd.dma_start(
                    out=wdb0, in_=prm["wd"][hs0 * P:(hs0 + 1) * P, :])
                wdbs[hs0] = wdb0
            with tc.tile_pool(name="pdw", bufs=3) as pdw, \
                 tc.tile_pool(name="pds", bufs=2) as pds, \
                 tc.tile_pool(name="abps", bufs=2, space="PSUM") as abpp:
                for hs in range(HSB):
                    w1b = pdw.tile([P, CCH, P], BF16, tag="w1b",
                                   name="w1b")
                    nc.sync.dma_start(out=w1b, in_=prm["w1"][hs])
                    w2b = pdw.tile([P, CCH, P], BF16, tag="w2b",
                                   name="w2b")
                    nc.sync.dma_start(out=w2b, in_=prm["w2"][hs])
                    ap_ = abpp.tile([P, QROWS], F32, tag="ap", name="ap")
                    bp_ = abpp.tile([P, QROWS], F32, tag="bp", name="bp")
                    # first block split by q-chunk so the MLP starts as
                    # soon as the first norm2 chain lands
                    qsl = ([slice(j * P, (j + 1) * P) for j in range(4)]
                           if hs <= 1 else [slice(0, QROWS)])
                    for sl in qsl:
                        for cc in range(CCH):
                            nc.tensor.matmul(
                                ap_[:, sl], lhsT=w1b[:, cc, :],
                                rhs=h2Tm[:, cc, sl],
                                start=(cc == 0), stop=(cc == CCH - 1))
                    for sl in qsl:
                        for cc in range(CCH):
                            nc.tensor.matmul(
                                bp_[:, sl], lhsT=w2b[:, cc, :],
                                rhs=h2Tm[:, cc, sl],
                                start=(cc == 0), stop=(cc == CCH - 1))
                    sT = pds.tile([P, QROWS], BF16, tag="sT", name="sT")
                    nc.scalar.activation(out=sT, in_=ap_, func=AF.Sigmoid)
                    nc.vector.tensor_tensor(
                        out=sT, in0=sT, in1=bp_, op=mybir.AluOpType.mult)
                    nc.vector.tensor_tensor(
                        out=mTm[:, hs, :], in0=sT, in1=ap_,
                        op=mybir.AluOpType.mult)

            if upto == "pd":
                _dummy_out()
                return
            # ---------------- Phase E: down proj + residual -------------
            with tc.tile_pool(name="peo", bufs=2) as peo, \
                 tc.tile_pool(name="x3ps", bufs=1, space="PSUM") as x3pp:
                x3p = [x3pp.tile([P, C], F32, tag=f"x3_{q}",
                                 name=f"x3_{q}") for q in range(QTILES)]
                for hs in range(HSB):
                    if hs in wdbs:
                        wdb = wdbs[hs]
                    else:
                        wdb = pew.tile([P, C], BF16, tag="wdb",
                                       name="wdb")
                        nc.gpsimd.dma_start(
                            out=wdb, in_=prm["wd"][hs * P:(hs + 1) * P, :])
                    for j in range(QTILES):
                        for half in range(2):
                            nc.tensor.matmul(
                                x3p[j][:, half * 512:(half + 1) * 512],
                                lhsT=mTm[:, hs, j * P:(j + 1) * P],
                                rhs=wdb[:, half * 512:(half + 1) * 512],
                                start=(hs == 0), stop=(hs == HSB - 1),
                            )
                for j in range(QTILES):
                    osb = peo.tile([P, C], F32, tag="osb", name="osb")
                    for half in range(2):
                        sl = slice(half * 512, (half + 1) * 512)
                        nc.vector.tensor_add(osb[:, sl], x3p[j][:, sl],
                                             x2sb[j][:, sl])
                        nc.sync.dma_start(
                            out=prm["out"][j * P:(j + 1) * P, sl],
                            in_=osb[:, sl])


def build_bass(upto="full", repeat=1):
    nc = bass.Bass("TRN2", target_bir_lowering=False, debug=False,
                   num_devices=8)
    prm = {}

    def inp(name, shape, dtype=F32):
        prm[name] = nc.declare_dram_parameter(name, list(shape), dtype,
                                              isOutput=False).ap()

    inp("xbf", (T, C), BF16)
    inp("xown", (QROWS, C))
    inp("cose", (T, 64), BF16)
    inp("sine", (T, 64), BF16)
    inp("smask", (TTILES, P, P), F8)
    inp("wq8", (CCH, P, C), F8)
    inp("wk8", (CCH, P, C), F8)
    inp("wv8", (CCH, P, C), F8)
    inp("wproj", (C, C), F8)
    inp("w1", (HSB, P, CCH, P), BF16)
    inp("w2", (HSB, P, CCH, P), BF16)
    inp("wd", (NHID, C), BF16)
    prm["out"] = nc.declare_dram_parameter("out", [QROWS, C], F32,
                                           isOutput=True).ap()
    with tile.TileContext(nc) as tc:
        for r in range(repeat):
            if r == repeat - 1:
                _emit(tc, nc, prm, upto=upto)
            else:
                sink = nc.dram_tensor(f"outsink{r}", [QROWS, C], F32).ap()
                _emit(tc, nc, dict(prm, out=sink), upto=upto)
    return nc


# ------------------------------------------------------------- host glue
def _rope_tables_expanded():
    """(T, 64) bf16 tables: col f = cos(t * theta_{f//2}); broadcast
    across the 16 heads on-chip (stride-0 AP)."""
    theta = (1.0 / (10000.0 ** (np.arange(0, HD, 2, dtype=np.float32)
                                / np.float32(HD)))).astype(np.float32)
    ang = np.outer(np.arange(T, dtype=np.float32), theta)  # (T, 32)
    cos = np.cos(ang).astype(np.float32)
    sin = np.sin(ang).astype(np.float32)
    cose = np.repeat(cos, 2, axis=1).astype(ml_dtypes.bfloat16)
    sine = np.repeat(sin, 2, axis=1).astype(ml_dtypes.bfloat16)
    return cose, sine


def _to_f8(w):
    return np.clip(w * WS, -240.0, 240.0).astype(ml_dtypes.float8_e4m3)


def _dr_layout(w):
    """(C, 1024) -> (CCH, 128, 1024)."""
    return np.ascontiguousarray(w.reshape(CCH, P, w.shape[1]))


def core_rows(c):
    t = c % 4
    tiles = [t, t + 4, t + 8, t + 12]
    return np.concatenate([np.arange(a * P, (a + 1) * P) for a in tiles])


def make_in_maps(x, y_mask, Wqkv, Wattn_proj, scale1, scale2, Wfc1, Wfc2,
                 Wmlp_proj):
    f = np.float32
    bf = ml_dtypes.bfloat16
    Wq = (scale1[:, None] * Wqkv[:, 0:C]).astype(f)
    Wk = (scale1[:, None] * Wqkv[:, C:2 * C]).astype(f)
    Wv = (scale1[:, None] * Wqkv[:, 2 * C:3 * C]).astype(f)
    wq8 = _dr_layout(_to_f8(Wq))
    wk8 = _dr_layout(_to_f8(Wk))
    wv8 = _dr_layout(_to_f8(Wv))
    wproj = _to_f8(Wattn_proj.astype(f))
    W1f = (scale2[:, None] * Wfc1).astype(f)
    W2f = (scale2[:, None] * Wfc2).astype(f)
    # (C, NHID) -> (HSB, P, CCH, P): w1[hs][p][cc][j] = W[cc*128+p, hs*128+j]
    w1 = np.ascontiguousarray(
        W1f.reshape(CCH, P, HSB, P).transpose(2, 1, 0, 3)).astype(bf)
    w2 = np.ascontiguousarray(
        W2f.reshape(CCH, P, HSB, P).transpose(2, 1, 0, 3)).astype(bf)
    wd = np.ascontiguousarray(Wmlp_proj.astype(f)).astype(bf)
    cose, sine = _rope_tables_expanded()

    kidx = np.arange(T)
    in_maps = []
    for c in range(8):
        b = c // 4
        c4 = c % 4
        # permute tiles: within each group of 4, own tile goes last
        tl = []
        for g in range(4):
            tl += [4 * g + r for r in range(4) if r != c4]
            tl.append(4 * g + c4)
        prows = np.concatenate(
            [np.arange(t * P, (t + 1) * P) for t in tl])
        ym = np.zeros(T, bool)
        ym[:64] = y_mask[b].astype(bool)
        # stripe masks: for k-tile kt (original tile tl[kt]), q-tile
        # j = kt//4 (original own tile c4+4j), 0/1 allowed
        smask = np.zeros((TTILES, P, P), np.float32)
        for kt in range(TTILES):
            j = kt // 4
            qabs = kidx[(c4 + 4 * j) * P:(c4 + 4 * j + 1) * P]
            kabs = kidx[tl[kt] * P:(tl[kt] + 1) * P]
            allowed = (kabs[:, None] <= qabs[None, :]) | (
                ym[kabs][:, None] & ym[qabs][None, :])
            smask[kt] = allowed.astype(np.float32)
        xperm = x[b][prows].astype(f)
        own_rows = np.concatenate(
            [np.arange((4 * j + 3) * P, (4 * j + 4) * P)
             for j in range(4)])
        in_maps.append({
            "xbf": np.ascontiguousarray(
                xperm.astype(ml_dtypes.bfloat16)),
            "xown": np.ascontiguousarray(xperm[own_rows]),
            "cose": np.ascontiguousarray(cose[prows]),
            "sine": np.ascontiguousarray(sine[prows]),
            "smask": smask.astype(ml_dtypes.float8_e4m3),
            "wq8": wq8, "wk8": wk8, "wv8": wv8,
            "wproj": wproj, "w1": w1, "w2": w2, "wd": wd,
        })
    return in_maps


_NC_CACHE = None


def kernel(**inputs):
    global _NC_CACHE
    in_maps = make_in_maps(**{k: np.asarray(v) for k, v in inputs.items()})
    if _NC_CACHE is None:
        _NC_CACHE = build_bass()
    res = run_bass_kernel_spmd(_NC_CACHE, in_maps, core_ids=list(range(8)))
    out = np.empty((B, T, C), np.float32)
    for c in range(8):
        out[c // 4, core_rows(c)] = res.results[c]["out"]
    return out

